# revision 3
# baseline (speedup 1.0000x reference)
"""AtomAttentionDecoder — 8-core Trainium2 kernel, v3.

Baseline shard formulation (compiles cleanly on neuronx-cc), with two
changes: single pmap over the 8 cores (one SPMD compile, one dispatch),
bf16 inputs for all large matmuls/einsums, and the pair-bias LayerNorm
folded algebraically: LN(p)*pg_i+pb_i @ wpb_i == (p @ Wc_i) * r_p + c_i
with Wc_i = (I - 11^T/16)(pg_i*wpb_i), c_i = pb_i@wpb_i, r_p the per-row
rsqrt(var). All three blocks' Wc are concatenated into one [16,12] matmul
done once, so p_lm is traversed once instead of three times.
Sharding: batch(2) x window-slices(4), halo 8 windows, no collectives.
"""

import os
import numpy as np


def _tune_cc_flags():
    mode = os.environ.get('CCTUNE', '')
    if not mode:
        return
    try:
        from concourse.compiler_utils import get_compiler_flags, set_compiler_flags
        flags = get_compiler_flags()
        out = []
        for f in flags:
            if f == '-O1' and 'O2' in mode:
                f = '-O2'
            if f.startswith('--tensorizer-options=') and 'noskip' in mode:
                f = '--tensorizer-options=--disable-dma-cast '
            if f.startswith('--internal-hlo2tensorizer-options=') and 'nothresh' in mode:
                continue
            out.append(f)
        set_compiler_flags(out)
        print('cc flags tuned:', mode)
    except Exception as e:
        print('cc flag tune failed:', e)


_tune_cc_flags()

B, NTOK, NATOM = 2, 512, 16384
C_TOKEN, C_ATOM, C_PAIR, C_S = 384, 128, 16, 384
NQ, NK, H, NB = 32, 128, 4, 3
DH = C_ATOM // H
NW = NATOM // NQ

WSLICES = 4
KEPT_W = NW // WSLICES
HALO_W = 8
LOC_W = KEPT_W + 2 * HALO_W
KEPT_A = KEPT_W * NQ
HALO_A = HALO_W * NQ
LOC_A = LOC_W * NQ

_jitted = None


def _build_shard_fn():
    import jax
    import jax.numpy as jnp
    bf16 = jnp.bfloat16
    f32 = jnp.float32

    def _ln(x, g=None, b=None, eps=1e-5):
        mu = jnp.mean(x, -1, keepdims=True)
        var = jnp.mean((x - mu) ** 2, -1, keepdims=True)
        xn = (x - mu) * jax.lax.rsqrt(var + eps)
        if g is not None:
            xn = xn * g
        if b is not None:
            xn = xn + b
        return xn

    def shard_fn(a, ef, plm, am, idx,
                 Wa, lnq_g, lnq_b, Wout,
                 ag_w, ag_b, ab_w, wq, bq, wk, wv, pg, pb, wpb, wg, wo,
                 sk_w, sk_b, tg_w, tg_b, tb_w, wt1, wt2, wto, tk_w, tk_b):
        q = (a.astype(bf16) @ Wa.astype(bf16)).astype(f32)   # [NTOK, C_ATOM]
        q = jnp.take(q, idx, axis=0)                         # [LOC_A, C_ATOM]
        q = q + ef
        amc = am[:, None]
        q = q * amc
        s = jnp.pad(ef, ((0, 0), (0, C_S - C_ATOM)))
        sn = _ln(s)
        snb = sn.astype(bf16)

        NBLK = NK // NQ  # 4

        def windows(t):
            pad = [(48, 80)] + [(0, 0)] * (t.ndim - 1)
            tp = jnp.pad(t, pad)
            blk = tp.reshape((LOC_W + NBLK, NQ) + t.shape[1:])
            w = jnp.stack([blk[j:j + LOC_W] for j in range(NBLK)], axis=1)
            return w.reshape((LOC_W, NK) + t.shape[1:])

        keymask = windows(am)                                # [LOC_W, NK]

        # pair bias for all 3 blocks in one pass over plm
        eye = jnp.eye(C_PAIR, dtype=f32)
        cen = eye - jnp.full((C_PAIR, C_PAIR), 1.0 / C_PAIR, f32)
        Wc = jnp.concatenate([cen @ (pg[i][:, None] * wpb[i])
                              for i in range(NB)], axis=1)   # [16, NB*H]
        cconst = jnp.stack([pb[i] @ wpb[i] for i in range(NB)])  # [NB, H]
        mu_p = jnp.mean(plm, -1, keepdims=True)
        var_p = jnp.mean(plm * plm, -1, keepdims=True) - mu_p * mu_p
        r_p = jax.lax.rsqrt(var_p + 1e-5)                    # [LOC_W,NQ,NK,1]
        raw = (plm.astype(bf16) @ Wc.astype(bf16)).astype(f32)   # [...,NB*H]
        braw = raw * r_p                                     # [LOC_W,NQ,NK,NB*H]

        def smm(w):   # sn @ w in bf16 -> f32
            return (snb @ w.astype(bf16)).astype(f32)

        x = q
        inv = 1.0 / np.sqrt(DH)
        for i in range(NB):
            xa = jax.nn.sigmoid(smm(ag_w[i]) + ag_b[i]) * _ln(x) + smm(ab_w[i])
            xab = xa.astype(bf16)
            qh = ((xab @ wq[i].astype(bf16)).astype(f32) + bq[i]) \
                .reshape(LOC_W, NQ, H, DH).astype(bf16)
            kh = (xab @ wk[i].astype(bf16)).reshape(LOC_A, H, DH)
            vh = (xab @ wv[i].astype(bf16)).reshape(LOC_A, H, DH)
            kw = windows(kh)                                 # [LOC_W, NK, H, DH]
            vw = windows(vh)
            bias = braw[..., i * H:(i + 1) * H] + cconst[i]  # [LOC_W, NQ, NK, H]
            scores = jnp.einsum('wqhd,wkhd->wqkh', qh, kw).astype(f32) * inv + bias
            scores = jnp.where(keymask[:, None, :, None] > 0, scores, -1e9)
            attn = jax.nn.softmax(scores, axis=2).astype(bf16)
            o = jnp.einsum('wqkh,wkhd->wqhd', attn, vw).reshape(LOC_A, C_ATOM)
            gate = jax.nn.sigmoid((xab @ wg[i].astype(bf16)).astype(f32))
            go = (gate * o.astype(f32)).astype(bf16)
            x = x + jax.nn.sigmoid(smm(sk_w[i]) + sk_b[i]) * \
                (go @ wo[i].astype(bf16)).astype(f32)
            xt = jax.nn.sigmoid(smm(tg_w[i]) + tg_b[i]) * _ln(x) + smm(tb_w[i])
            xtb = xt.astype(bf16)
            h1 = (xtb @ wt1[i].astype(bf16)).astype(f32)
            h2 = (xtb @ wt2[i].astype(bf16)).astype(f32)
            hsw = (jax.nn.silu(h1) * h2).astype(bf16)
            x = x + jax.nn.sigmoid(smm(tk_w[i]) + tk_b[i]) * \
                (hsw @ wto[i].astype(bf16)).astype(f32)

        x = x * amc
        r = _ln(x, lnq_g, lnq_b) @ Wout                      # [LOC_A, 3]
        return r[HALO_A:HALO_A + KEPT_A]

    return jax.pmap(shard_fn, devices=jax.devices()[:8])


def _pad_slice(arr, lo, hi):
    n = arr.shape[0]
    lo_pad = max(0, -lo)
    hi_pad = max(0, hi - n)
    core = arr[max(lo, 0):min(hi, n)]
    if lo_pad or hi_pad:
        pad = [(lo_pad, hi_pad)] + [(0, 0)] * (arr.ndim - 1)
        core = np.pad(core, pad)
    return core


WNAMES = ['Wa', 'lnq_g', 'lnq_b', 'Wout',
          'ag_w', 'ag_b', 'ab_w', 'wq', 'bq', 'wk', 'wv', 'pg', 'pb',
          'wpb', 'wg', 'wo', 'sk_w', 'sk_b', 'tg_w', 'tg_b', 'tb_w',
          'wt1', 'wt2', 'wto', 'tk_w', 'tk_b']


def stage_args(inputs):
    """Build stacked [8, ...] pmap args from full inputs (host side)."""
    weights = [np.asarray(inputs[k], np.float32) for k in WNAMES]
    a = np.asarray(inputs['a'], np.float32)
    ef = np.asarray(inputs['extra_feats'], np.float32)
    plm = np.asarray(inputs['p_lm'], np.float32)
    am = np.asarray(inputs['atom_mask'], np.float32)
    idx = np.asarray(inputs['atom_to_token_idx'], np.int32)

    sa, sef, splm, sam, sidx = [], [], [], [], []
    for c in range(8):
        b, ws = divmod(c, WSLICES)
        a0 = ws * KEPT_A - HALO_A
        a1 = ws * KEPT_A + KEPT_A + HALO_A
        w0 = ws * KEPT_W - HALO_W
        w1 = ws * KEPT_W + KEPT_W + HALO_W
        sa.append(a[b])
        sef.append(_pad_slice(ef[b], a0, a1))
        splm.append(_pad_slice(plm[b], w0, w1))
        sam.append(_pad_slice(am[b], a0, a1))
        sidx.append(np.clip(_pad_slice(idx[b], a0, a1), 0, NTOK - 1))
    args = [np.stack(sa), np.stack(sef), np.stack(splm), np.stack(sam),
            np.stack(sidx)]
    args += [np.broadcast_to(w, (8,) + w.shape).copy() for w in weights]
    return args


def kernel(**inputs) -> np.ndarray:
    global _jitted
    if _jitted is None:
        _jitted = _build_shard_fn()
    f = _jitted

    outs = np.asarray(f(*stage_args(inputs)))   # [8, KEPT_A, 3]
    full = np.empty((B, NATOM, 3), np.float32)
    for c in range(8):
        b, ws = divmod(c, WSLICES)
        full[b, ws * KEPT_A:(ws + 1) * KEPT_A] = outs[c]
    return full


# revision 4
# speedup vs baseline: 1.6581x; 1.6581x over previous
"""AtomAttentionDecoder — 8-core Trainium2 kernel, v7.

v7 + full-bf16: the residual stream, pair-bias table and score tensors
are bf16 as well (LN statistics still accumulate in f32). Minimizes HBM
bytes end to end; only the tiny final head runs in f32.
"""

import os
import numpy as np

B, NTOK, NATOM = 2, 512, 16384
C_TOKEN, C_ATOM, C_PAIR, C_S = 384, 128, 16, 384
NQ, NK, H, NB = 32, 128, 4, 3
DH = C_ATOM // H
NW = NATOM // NQ

WSLICES = 4
KEPT_W = NW // WSLICES
HALO_W = 8
LOC_W = KEPT_W + 2 * HALO_W
KEPT_A = KEPT_W * NQ
HALO_A = HALO_W * NQ
LOC_A = LOC_W * NQ

_jitted = None


def _build_shard_fn():
    import jax
    import jax.numpy as jnp
    bf16 = jnp.bfloat16
    f32 = jnp.float32

    def _ln(x, eps=1e-5):
        mu = jnp.mean(x, -1, keepdims=True)
        var = jnp.mean((x - mu) ** 2, -1, keepdims=True)
        return (x - mu) * jax.lax.rsqrt(var + eps)

    def _lnb(x, eps=1e-5):
        # bf16 in/out, f32 reduction accumulators
        mu = jnp.mean(x, -1, keepdims=True, dtype=f32).astype(bf16)
        d = x - mu
        var = jnp.mean((d * d).astype(f32), -1, keepdims=True)
        return d * jax.lax.rsqrt(var + eps).astype(bf16)

    def shard_fn(a, ef, plm, am, idx,
                 Wa, lnq_g, lnq_b, Wout,
                 Wgcat, bgcat, Wbcat, wqkvg, bqs, wo, wt12, wto, Wc, ccn):
        # a, ef, plm, am and all W* except lnq/Wout/ccn arrive bf16
        q = a @ Wa                                           # bf16 [NTOK,128]
        q = jnp.take(q, idx, axis=0)                         # [LOC_A, 128]
        q = (q + ef) * am[:, None]                           # bf16
        s = jnp.pad(ef, ((0, 0), (0, C_S - C_ATOM)))
        sn = _ln(s)                                          # bf16 [LOC_A,384]

        G = jax.nn.sigmoid(sn @ Wgcat + bgcat)               # bf16 [.,1536]
        Bc = sn @ Wbcat                                      # bf16 [.,768]

        NBLK = NK // NQ  # 4

        def windows(t):
            pad = [(48, 80)] + [(0, 0)] * (t.ndim - 1)
            tp = jnp.pad(t, pad)
            blk = tp.reshape((LOC_W + NBLK, NQ) + t.shape[1:])
            w = jnp.stack([blk[j:j + LOC_W] for j in range(NBLK)], axis=1)
            return w.reshape((LOC_W, NK) + t.shape[1:])

        keymask = windows(am)                                # bf16 [LOC_W,NK]
        negmask = (keymask <= 0).astype(f32) * (-1e9)

        # pair bias, one pass (bf16 plm, f32 accumulate)
        mu_p = jnp.mean(plm, -1, keepdims=True)
        msq = jnp.mean(plm * plm, -1, keepdims=True)
        var_p = (msq - mu_p * mu_p).astype(f32)
        r_p = jax.lax.rsqrt(var_p + 1e-5)                    # f32
        raw = jnp.einsum('wqkc,ch->wqkh', plm, Wc,
                         preferred_element_type=f32)         # [.,NB*H] f32
        addend = ccn[None, None, None, :] + negmask[:, None, :, None]
        braw = (raw * r_p + addend).astype(bf16)

        x = q
        for i in range(NB):
            gA = G[:, (4 * i + 0) * 128:(4 * i + 1) * 128]
            gS = G[:, (4 * i + 1) * 128:(4 * i + 2) * 128]
            gT = G[:, (4 * i + 2) * 128:(4 * i + 3) * 128]
            gK = G[:, (4 * i + 3) * 128:(4 * i + 4) * 128]
            bA = Bc[:, (2 * i + 0) * 128:(2 * i + 1) * 128]
            bT = Bc[:, (2 * i + 1) * 128:(2 * i + 2) * 128]

            xab = gA * _lnb(x) + bA                          # bf16
            qkvg = xab @ wqkvg[i]                            # bf16 [LOC_A,512]
            qh = (qkvg[:, 0:128] + bqs[i]).reshape(LOC_W, NQ, H, DH)
            kh = qkvg[:, 128:256].reshape(LOC_A, H, DH)
            vh = qkvg[:, 256:384].reshape(LOC_A, H, DH)
            gate = jax.nn.sigmoid(qkvg[:, 384:512])          # bf16
            kw = windows(kh)                                 # bf16
            vw = windows(vh)
            scores = jnp.einsum('wqhd,wkhd->wqkh', qh, kw) \
                + braw[..., i * H:(i + 1) * H]
            e = jnp.exp(scores)                              # bf16
            denom = jnp.sum(e, axis=2, dtype=f32) + 1e-30    # [LOC_W,NQ,H]
            recip = (1.0 / denom).astype(bf16)
            o = jnp.einsum('wqkh,wkhd->wqhd', e, vw)         # bf16
            o = o * recip[..., None]
            go = gate * o.reshape(LOC_A, C_ATOM)
            x = x + gS * jnp.einsum('ac,cd->ad', go, wo[i])

            xtb = gT * _lnb(x) + bT                          # bf16
            h12 = xtb @ wt12[i]                              # bf16 [LOC_A,512]
            hsw = jax.nn.silu(h12[:, :256]) * h12[:, 256:]   # bf16
            x = x + gK * jnp.einsum('ac,cd->ad', hsw, wto[i])

        x = (x * am[:, None]).astype(f32)
        r = (_ln(x) * lnq_g + lnq_b) @ Wout                  # f32 [LOC_A,3]
        return r[HALO_A:HALO_A + KEPT_A]

    return jax.pmap(shard_fn, devices=jax.devices()[:8])


def _pad_slice(arr, lo, hi):
    n = arr.shape[0]
    lo_pad = max(0, -lo)
    hi_pad = max(0, hi - n)
    core = arr[max(lo, 0):min(hi, n)]
    if lo_pad or hi_pad:
        pad = [(lo_pad, hi_pad)] + [(0, 0)] * (arr.ndim - 1)
        core = np.pad(core, pad)
    return core


def _prep_weights(inputs):
    """Host-side weight consolidation (numpy); big ones cast to bf16."""
    import ml_dtypes
    b16 = ml_dtypes.bfloat16
    g = lambda k: np.asarray(inputs[k], np.float32)
    inv = 1.0 / np.sqrt(DH)
    Wgcat = np.concatenate([np.concatenate(
        [g('ag_w')[i], g('sk_w')[i], g('tg_w')[i], g('tk_w')[i]], axis=1)
        for i in range(NB)], axis=1).astype(b16)            # [384, 12*128]
    bgcat = np.concatenate([np.concatenate(
        [g('ag_b')[i], g('sk_b')[i], g('tg_b')[i], g('tk_b')[i]])
        for i in range(NB)]).astype(b16)                    # [12*128]
    Wbcat = np.concatenate([np.concatenate(
        [g('ab_w')[i], g('tb_w')[i]], axis=1)
        for i in range(NB)], axis=1).astype(b16)            # [384, 6*128]
    wqkvg = np.stack([np.concatenate(
        [g('wq')[i] * inv, g('wk')[i], g('wv')[i], g('wg')[i]], axis=1)
        for i in range(NB)]).astype(b16)                    # [NB, 128, 512]
    bqs = np.stack([g('bq')[i] * inv for i in range(NB)]).astype(b16)
    wt12 = np.stack([np.concatenate(
        [g('wt1')[i], g('wt2')[i]], axis=1) for i in range(NB)]).astype(b16)
    cen = np.eye(C_PAIR, dtype=np.float32) - 1.0 / C_PAIR
    Wc = np.concatenate([cen @ (g('pg')[i][:, None] * g('wpb')[i])
                         for i in range(NB)], axis=1).astype(b16)  # [16,NB*H]
    ccn = np.concatenate([g('pb')[i] @ g('wpb')[i] for i in range(NB)])
    return [g('Wa').astype(b16), g('lnq_g'), g('lnq_b'), g('Wout'),
            Wgcat, bgcat, Wbcat, wqkvg, bqs, g('wo').astype(b16), wt12,
            g('wto').astype(b16), Wc, ccn.astype(np.float32)]


def stage_args(inputs):
    """Build stacked [8, ...] pmap args from full inputs (host side)."""
    import ml_dtypes
    b16 = ml_dtypes.bfloat16
    weights = _prep_weights(inputs)
    a = np.asarray(inputs['a'], np.float32).astype(b16)
    ef = np.asarray(inputs['extra_feats'], np.float32).astype(b16)
    plm = np.asarray(inputs['p_lm'], np.float32).astype(b16)
    am = np.asarray(inputs['atom_mask'], np.float32).astype(b16)
    idx = np.asarray(inputs['atom_to_token_idx'], np.int32)

    sa, sef, splm, sam, sidx = [], [], [], [], []
    for c in range(8):
        b, ws = divmod(c, WSLICES)
        a0 = ws * KEPT_A - HALO_A
        a1 = ws * KEPT_A + KEPT_A + HALO_A
        w0 = ws * KEPT_W - HALO_W
        w1 = ws * KEPT_W + KEPT_W + HALO_W
        sa.append(a[b])
        sef.append(_pad_slice(ef[b], a0, a1))
        splm.append(_pad_slice(plm[b], w0, w1))
        sam.append(_pad_slice(am[b], a0, a1))
        sidx.append(np.clip(_pad_slice(idx[b], a0, a1), 0, NTOK - 1))
    args = [np.stack(sa), np.stack(sef), np.stack(splm), np.stack(sam),
            np.stack(sidx)]
    args += [np.broadcast_to(w, (8,) + w.shape).copy() for w in weights]
    return args


def kernel(**inputs) -> np.ndarray:
    global _jitted
    if _jitted is None:
        _jitted = _build_shard_fn()
    f = _jitted

    outs = np.asarray(f(*stage_args(inputs)))   # [8, KEPT_A, 3]
    full = np.empty((B, NATOM, 3), np.float32)
    for c in range(8):
        b, ws = divmod(c, WSLICES)
        full[b, ws * KEPT_A:(ws + 1) * KEPT_A] = outs[c]
    return full
